# revision 1
# baseline (speedup 1.0000x reference)
"""Trainium2 Bass kernel for nn_NodeNet (GNN message passing).

Reference computation:
    bo = Ro.T @ X            [E, D]   (gather per-edge source feats)
    bi = Ri.T @ X            [E, D]
    mi = (Ri * e.T) @ bo     [N, D]   (edge-weighted scatter-add)
    mo = (Ro * e.T) @ bi     [N, D]
    M  = [mi, mo, X]         [N, 3D]
    y  = sigmoid(tanh(M @ W1 + b1) @ W2 + b2)

Strategy (8 NeuronCores, edge-sharded):
  - Shard the edge axis E across 8 cores (3072 edges each). Each core
    computes partial mi/mo from its edge shard; AllReduce the [8, N]
    partials; every core runs the tiny MLP and writes the full output.
  - The PE contracts over the partition axis, so the gather needs the
    incidence matrices with N on partitions while the scatter needs E on
    partitions. We upload both layouts in fp16 (host casts/transposes),
    which costs the same HBM bytes as a single fp32 read.
  - e-weighting is applied to the small [E, D] gathered tensors, not the
    big matrices:  mi = Ri @ (e * bo),  mo = Ro @ (e * bi).
"""

import os
import numpy as np

N = 8192
E = 24576
D = 4
H = 100
CORES = 8
ESH = E // CORES          # 3072 edges per core
NCH = N // 128            # 64 node chunks (gather contraction steps)
ECH = ESH // 128          # 24 edge chunks per core
NSLAB = 512               # node-slab width for scatter / MLP
NS = N // NSLAB           # 16 node slabs

_last_exec_time_ns = None
_cached = {}
# fp8 storage for one-hot incidence matrices: flipped on after HW validation.
_FP8_VALIDATED = False


def _build(collective: bool = True, phases: int = 4, r_dtype: str = "float16"):
    # phases: 1=gather only, 2=+scatter, 3=+allreduce, 4=+mlp (full)
    # r_dtype: storage dtype for the big incidence matrices. float8e4 is
    # exact for one-hot matrices and halves HBM traffic.
    import concourse.bass as bass
    import concourse.bacc as bacc
    import concourse.mybir as mybir
    import concourse.tile as tile

    f32 = mybir.dt.float32
    f16 = mybir.dt.float16
    fR = getattr(mybir.dt, r_dtype)

    nc = bacc.Bacc(
        "TRN2",
        target_bir_lowering=False,
        debug=False,
        num_devices=CORES if collective else 1,
    )

    Ri_nat = nc.dram_tensor("Ri_nat", [N, ESH], fR, kind="ExternalInput").ap()
    Ro_nat = nc.dram_tensor("Ro_nat", [N, ESH], fR, kind="ExternalInput").ap()
    RiT = nc.dram_tensor("RiT", [ESH, N], fR, kind="ExternalInput").ap()
    RoT = nc.dram_tensor("RoT", [ESH, N], fR, kind="ExternalInput").ap()
    Xg = nc.dram_tensor("Xg", [128, NCH * D], f16, kind="ExternalInput").ap()
    XT = nc.dram_tensor("XT", [D, N], f32, kind="ExternalInput").ap()
    esh = nc.dram_tensor("esh", [128, ECH], f32, kind="ExternalInput").ap()
    W1 = nc.dram_tensor("W1", [3 * D, H], f32, kind="ExternalInput").ap()
    b1 = nc.dram_tensor("b1", [H, 1], f32, kind="ExternalInput").ap()
    W2 = nc.dram_tensor("W2", [H, 1], f32, kind="ExternalInput").ap()
    b2 = nc.dram_tensor("b2", [1, 1], f32, kind="ExternalInput").ap()
    y = nc.dram_tensor("y", [1, N], f32, kind="ExternalOutput").ap()

    with tile.TileContext(nc) as tc:
        with (
            tc.tile_pool(name="const", bufs=1) as const,
            tc.tile_pool(name="gslab", bufs=3) as gslab_pool,
            tc.tile_pool(name="sslab", bufs=2) as sslab_pool,
            tc.tile_pool(name="small", bufs=1) as small,
            tc.tile_pool(name="mlp", bufs=2) as mlp_pool,
            tc.tile_pool(name="psA", bufs=2, space="PSUM") as psA,
            tc.tile_pool(name="psB", bufs=2, space="PSUM") as psB,
            tc.tile_pool(name="dram", bufs=1, space="DRAM") as dram,
        ):
            # ---- resident small tensors ----
            Xg_sb = const.tile([128, NCH * D], f16)
            nc.sync.dma_start(out=Xg_sb[:], in_=Xg[:])
            e_sb = const.tile([128, ECH], f32)
            nc.sync.dma_start(out=e_sb[:], in_=esh[:])
            W1_sb = const.tile([3 * D, H], f32)
            nc.sync.dma_start(out=W1_sb[:], in_=W1[:])
            b1_sb = const.tile([H, 1], f32)
            nc.sync.dma_start(out=b1_sb[:], in_=b1[:])
            W2_sb = const.tile([H, 1], f32)
            nc.sync.dma_start(out=W2_sb[:], in_=W2[:])
            b2_sb = const.tile([1, 1], f32)
            nc.sync.dma_start(out=b2_sb[:], in_=b2[:])

            # M.T rows: 0-3 mi, 4-7 mo, 8-11 X
            MT_sb = small.tile([3 * D, N], f32)
            nc.sync.dma_start(out=MT_sb[2 * D : 3 * D, :], in_=XT[:])

            # ---- phase 1: gather  b = R.T @ X  -> [ESH, D], e on partitions
            # bv tiles: [128, 12] fp16 per edge chunk, zero-padded so the
            # scatter matmul writes disjoint rows of one [12, NSLAB] psum.
            #   bvi[ech][:, 0:4] = e * bo   (pairs with RiT -> mi rows 0-3)
            #   bvo[ech][:, 4:8] = e * bi   (pairs with RoT -> mo rows 4-7)
            bvi = small.tile([128, ECH * 12], f16)
            bvo = small.tile([128, ECH * 12], f16)
            nc.vector.memset(bvi[:], 0.0)
            nc.vector.memset(bvo[:], 0.0)

            # NOTE: start=True clears has_written for the WHOLE psum bank, so
            # independent accumulation regions cannot share a bank across a
            # long accumulation. Instead: one-shot matmuls per n-chunk into a
            # fresh psum tile, accumulated into fp32 SBUF with DVE adds.
            for Rnat, dst, col0, acc_tag in (
                (Ri_nat, bvo, 4, "bacc_i"),
                (Ro_nat, bvi, 0, "bacc_o"),
            ):
                bacc = small.tile([128, ECH * D], f32, tag=acc_tag)
                for nch in range(NCH):
                    slab = gslab_pool.tile([128, ESH], fR, tag="gs")
                    nc.sync.dma_start(
                        out=slab[:], in_=Rnat[nch * 128 : (nch + 1) * 128, :]
                    )
                    bpsum = psA.tile([128, ECH * D], f32, tag="gather_ps")
                    for ech in range(ECH):
                        nc.tensor.matmul(
                            bpsum[:, ech * D : (ech + 1) * D],
                            lhsT=slab[:, ech * 128 : (ech + 1) * 128],
                            rhs=Xg_sb[:, nch * D : (nch + 1) * D],
                            start=True,
                            stop=True,
                        )
                    if nch == 0:
                        nc.vector.tensor_copy(bacc[:], bpsum[:])
                    else:
                        nc.vector.tensor_add(bacc[:], bacc[:], bpsum[:])
                # bv = e * b, cast to fp16
                for ech in range(ECH):
                    nc.vector.tensor_scalar_mul(
                        dst[:, ech * 12 + col0 : ech * 12 + col0 + D],
                        bacc[:, ech * D : (ech + 1) * D],
                        e_sb[:, ech : ech + 1],
                    )

            # ---- phase 2: scatter  miT/moT = bv.T @ RT  -> psum [12, NSLAB]
            RiT3 = RiT.rearrange("(ec p) n -> p ec n", p=128)
            RoT3 = RoT.rearrange("(ec p) n -> p ec n", p=128)
            for ns in range(NS if phases >= 2 else 0):
                mpsum = psB.tile([3 * D, NSLAB], f32, tag="scat_ps")
                first = True
                for RT3, bv, stag in ((RiT3, bvi, "ssi"), (RoT3, bvo, "sso")):
                    tslab = sslab_pool.tile([128, ECH, NSLAB], fR, tag=stag)
                    nc.sync.dma_start(
                        out=tslab[:],
                        in_=RT3[:, :, ns * NSLAB : (ns + 1) * NSLAB],
                    )
                    for ech in range(ECH):
                        nc.tensor.matmul(
                            mpsum[:],
                            lhsT=bv[:, ech * 12 : (ech + 1) * 12],
                            rhs=tslab[:, ech, :],
                            start=first,
                            stop=(bv is bvo and ech == ECH - 1),
                        )
                        first = False
                nc.vector.tensor_copy(
                    MT_sb[0 : 2 * D, ns * NSLAB : (ns + 1) * NSLAB],
                    mpsum[0 : 2 * D, :],
                )

            # ---- phase 3: all-reduce partial mi/mo across the 8 cores ----
            if collective and phases >= 3:
                ar_in = dram.tile([2 * D, N], f32)
                ar_out = dram.tile([2 * D, N], f32, addr_space="Shared")
                nc.gpsimd.dma_start(out=ar_in[:], in_=MT_sb[0 : 2 * D, :])
                nc.gpsimd.collective_compute(
                    "AllReduce",
                    mybir.AluOpType.add,
                    replica_groups=[list(range(CORES))],
                    ins=[ar_in.opt()],
                    outs=[ar_out.opt()],
                )
                nc.gpsimd.dma_start(out=MT_sb[0 : 2 * D, :], in_=ar_out[:])

            # ---- phase 4: MLP  y = sigmoid(tanh(M @ W1 + b1) @ W2 + b2) ----
            for ns in range(NS if phases >= 4 else 0):
                hpsum = psB.tile([H, NSLAB], f32, tag="h_ps")
                nc.tensor.matmul(
                    hpsum[:],
                    lhsT=W1_sb[:],
                    rhs=MT_sb[:, ns * NSLAB : (ns + 1) * NSLAB],
                    start=True,
                    stop=True,
                )
                h_sb = mlp_pool.tile([H, NSLAB], f32, tag="h_sb")
                nc.scalar.activation(
                    h_sb[:], hpsum[:], mybir.ActivationFunctionType.Tanh,
                    bias=b1_sb[:],
                )
                ypsum = psB.tile([1, NSLAB], f32, tag="y_ps")
                nc.tensor.matmul(
                    ypsum[:], lhsT=W2_sb[:], rhs=h_sb[:], start=True, stop=True
                )
                y_sb = mlp_pool.tile([1, NSLAB], f32, tag="y_sb")
                nc.scalar.activation(
                    y_sb[:], ypsum[:], mybir.ActivationFunctionType.Sigmoid,
                    bias=b2_sb[:],
                )
                nc.sync.dma_start(
                    out=y[:, ns * NSLAB : (ns + 1) * NSLAB], in_=y_sb[:]
                )

    nc.compile()
    return nc


def _get_nc(r_dtype: str = "float16"):
    if r_dtype not in _cached:
        _cached[r_dtype] = _build(r_dtype=r_dtype)
    return _cached[r_dtype]


def _is_binary(a, sample=65536):
    flat = a.reshape(-1)
    s = flat[:: max(1, flat.size // sample)]
    if not np.all((s == 0.0) | (s == 1.0)):
        return False
    return bool(np.all((flat == 0.0) | (flat == 1.0)))


def _r_np_dtype(r_dtype: str):
    if r_dtype == "float16":
        return np.float16
    import ml_dtypes
    return ml_dtypes.float8_e4m3


def _prepare_in_maps(X, e, Ri, Ro, W1, b1, W2, b2, r_dtype: str = "float16"):
    X = np.asarray(X, dtype=np.float32)
    e = np.asarray(e, dtype=np.float32)
    W1 = np.asarray(W1, dtype=np.float32)
    b1 = np.asarray(b1, dtype=np.float32)
    W2 = np.asarray(W2, dtype=np.float32)
    b2 = np.asarray(b2, dtype=np.float32)

    rdt = _r_np_dtype(r_dtype)
    Ri16 = np.asarray(Ri, dtype=np.float32).astype(rdt)
    Ro16 = np.asarray(Ro, dtype=np.float32).astype(rdt)
    RiT16 = np.ascontiguousarray(Ri16.T)   # [E, N]
    RoT16 = np.ascontiguousarray(Ro16.T)

    X16 = X.astype(np.float16)
    # Xg[p, nch*D + d] = X[nch*128 + p, d]
    Xg = np.ascontiguousarray(
        X16.reshape(NCH, 128, D).transpose(1, 0, 2).reshape(128, NCH * D)
    )
    XT = np.ascontiguousarray(X.T)         # [D, N] fp32

    b1c = np.ascontiguousarray(b1.reshape(H, 1))
    b2c = np.ascontiguousarray(b2.reshape(1, 1))
    W1c = np.ascontiguousarray(W1)
    W2c = np.ascontiguousarray(W2.reshape(H, 1))

    in_maps = []
    for c in range(CORES):
        sh = slice(c * ESH, (c + 1) * ESH)
        # esh[p, ech] = e[c*ESH + ech*128 + p]
        e_c = np.ascontiguousarray(
            e.reshape(-1)[sh].reshape(ECH, 128).T
        ).astype(np.float32)
        in_maps.append(
            {
                "Ri_nat": np.ascontiguousarray(Ri16[:, sh]),
                "Ro_nat": np.ascontiguousarray(Ro16[:, sh]),
                "RiT": RiT16[sh],
                "RoT": RoT16[sh],
                "Xg": Xg,
                "XT": XT,
                "esh": e_c,
                "W1": W1c,
                "b1": b1c,
                "W2": W2c,
                "b2": b2c,
            }
        )
    return in_maps


def kernel(**inputs) -> np.ndarray:
    global _last_exec_time_ns
    from concourse import bass_utils

    Ri = np.asarray(inputs["Ri"], dtype=np.float32)
    Ro = np.asarray(inputs["Ro"], dtype=np.float32)
    # fp8 storage is exact for one-hot incidence matrices; otherwise fp16.
    if os.environ.get("KERNEL_R_DTYPE"):
        r_dtype = os.environ["KERNEL_R_DTYPE"]
    elif _FP8_VALIDATED and _is_binary(Ri) and _is_binary(Ro):
        r_dtype = "float8e4"
    else:
        r_dtype = "float16"

    nc = _get_nc(r_dtype)
    in_maps = _prepare_in_maps(
        inputs["X"], inputs["e"], Ri, Ro,
        inputs["W1"], inputs["b1"], inputs["W2"], inputs["b2"],
        r_dtype=r_dtype,
    )
    trace = os.environ.get("KERNEL_TRACE", "") == "1"
    res = bass_utils.run_bass_kernel_spmd(
        nc, in_maps, core_ids=list(range(CORES)), trace=trace
    )
    _last_exec_time_ns = res.exec_time_ns
    out = np.asarray(res.results[0]["y"], dtype=np.float32).reshape(N, 1)
    return out



# revision 3
# speedup vs baseline: 3.3031x; 3.3031x over previous
"""Trainium2 Bass kernel for nn_NodeNet (GNN message passing).

Reference computation:
    bo = Ro.T @ X            [E, D]   (gather per-edge source feats)
    bi = Ri.T @ X            [E, D]
    mi = (Ri * e.T) @ bo     [N, D]   (edge-weighted scatter-add)
    mo = (Ro * e.T) @ bi     [N, D]
    M  = [mi, mo, X]         [N, 3D]
    y  = sigmoid(tanh(M @ W1 + b1) @ W2 + b2)

Fast path (sparse, 8 NeuronCores, node-range sharded):
  Ri/Ro are one-hot incidence matrices, so each edge k is fully described
  by (idx_i[k], idx_o[k], e[k]).  Shipping the dense [N, E] matrices to
  the device costs ~1.6 GB per execution; shipping indices costs ~3 MB.

  - Host extracts idx_i/idx_o (exact-one-hot validated; dense fallback
    otherwise) and shards EDGES BY TARGET-NODE RANGE: core c owns nodes
    [c*1024, (c+1)*1024) and receives the edges whose target (idx_i for
    mi, idx_o for mo) lands in its range.  Partial sums never cross
    cores, so NO collective is needed.
  - On device, per pass (mi / mo):
      1. one indirect DMA gathers X[src] rows for all edges ([128, T*4]),
      2. DVE scales rows by e,
      3. per 512-node slab, DVE builds one-hot [128, 512] scatter tiles
         (is_equal against an iota) and the PE accumulates
         psum[8, 512] += v[128, 8].T @ onehot  (mi in rows 0-3, mo in
         rows 4-7 via zero-padded lhsT columns).
  - The tiny MLP runs per slab exactly as in the dense kernel.
"""

import os
import numpy as np

N = 8192
E = 24576
D = 4
H = 100
CORES = 8
NPC = N // CORES          # 1024 nodes per core
SLAB = 512                # psum-width node slab
SLABS = NPC // SLAB       # 2 slabs per core

# dense-fallback constants
ESH = E // CORES
NCH = N // 128
ECH = ESH // 128
NSLAB = 512
NS = N // NSLAB

_last_exec_time_ns = None
_cached = {}
_cached_sparse = {}


# --------------------------------------------------------------------------
# sparse fast path
# --------------------------------------------------------------------------

def _build_sparse(T0: int, T1: int):
    import concourse.bass as bass
    import concourse.bacc as bacc
    import concourse.mybir as mybir
    import concourse.tile as tile

    f32 = mybir.dt.float32
    i32 = mybir.dt.int32
    TT = T0 + T1

    nc = bacc.Bacc(
        "TRN2",
        target_bir_lowering=False,
        debug=False,
        num_devices=CORES,
    )

    Xtab = nc.dram_tensor("Xtab", [N, D], f32, kind="ExternalInput").ap()
    XTl = nc.dram_tensor("XTl", [D, NPC], f32, kind="ExternalInput").ap()
    srcA = nc.dram_tensor("srcA", [128, TT], i32, kind="ExternalInput").ap()
    posA = nc.dram_tensor("posA", [128, TT], f32, kind="ExternalInput").ap()
    ewA = nc.dram_tensor("ewA", [128, TT], f32, kind="ExternalInput").ap()
    srcB = nc.dram_tensor("srcB", [128, TT], i32, kind="ExternalInput").ap()
    posB = nc.dram_tensor("posB", [128, TT], f32, kind="ExternalInput").ap()
    ewB = nc.dram_tensor("ewB", [128, TT], f32, kind="ExternalInput").ap()
    W1 = nc.dram_tensor("W1", [3 * D, H], f32, kind="ExternalInput").ap()
    b1 = nc.dram_tensor("b1", [H, 1], f32, kind="ExternalInput").ap()
    W2 = nc.dram_tensor("W2", [H, 1], f32, kind="ExternalInput").ap()
    b2 = nc.dram_tensor("b2", [1, 1], f32, kind="ExternalInput").ap()
    y = nc.dram_tensor("y", [1, NPC], f32, kind="ExternalOutput").ap()

    with tile.TileContext(nc) as tc:
        with (
            tc.tile_pool(name="const", bufs=1) as const,
            tc.tile_pool(name="oh", bufs=4) as ohpool,
            tc.tile_pool(name="mlp", bufs=2) as mlp_pool,
            tc.tile_pool(name="psS", bufs=2, space="PSUM") as psS,
            tc.tile_pool(name="psH", bufs=2, space="PSUM") as psH,
            tc.tile_pool(name="psY", bufs=2, space="PSUM") as psY,
        ):
            # ---- resident small tensors ----
            W1_sb = const.tile([3 * D, H], f32)
            nc.sync.dma_start(out=W1_sb[:], in_=W1[:])
            b1_sb = const.tile([H, 1], f32)
            nc.sync.dma_start(out=b1_sb[:], in_=b1[:])
            W2_sb = const.tile([H, 1], f32)
            nc.sync.dma_start(out=W2_sb[:], in_=W2[:])
            b2_sb = const.tile([1, 1], f32)
            nc.sync.dma_start(out=b2_sb[:], in_=b2[:])

            # M.T rows: 0-3 mi, 4-7 mo, 8-11 X
            MT_sb = const.tile([3 * D, NPC], f32)
            nc.sync.dma_start(out=MT_sb[2 * D : 3 * D, :], in_=XTl[:])

            iota_i = const.tile([128, SLAB], i32)
            nc.gpsimd.iota(
                iota_i[:], pattern=[[1, SLAB]], base=0, channel_multiplier=0
            )
            iota_f = const.tile([128, SLAB], f32)
            nc.vector.tensor_copy(iota_f[:], iota_i[:])

            # ---- per pass: gather + e-scale ----
            passes = []
            for pname, srcT, posT, ewT, col0 in (
                ("A", srcA, posA, ewA, 0),
                ("B", srcB, posB, ewB, D),
            ):
                src_sb = const.tile([128, TT], i32, tag=f"src{pname}")
                nc.sync.dma_start(out=src_sb[:], in_=srcT[:])
                pos_sb = const.tile([128, TT], f32, tag=f"pos{pname}")
                nc.sync.dma_start(out=pos_sb[:], in_=posT[:])
                ew_sb = const.tile([128, TT], f32, tag=f"ew{pname}")
                nc.sync.dma_start(out=ew_sb[:], in_=ewT[:])

                # NOTE: HW processes ONE offset per partition per indirect
                # DMA (multi-column offset tiles silently misbehave), so
                # gather tile-by-tile with [128, 1] offsets.
                gath = const.tile([128, TT * D], f32, tag=f"gath{pname}")
                for t in range(TT):
                    nc.gpsimd.indirect_dma_start(
                        out=gath[:, t * D : (t + 1) * D],
                        out_offset=None,
                        in_=Xtab[:],
                        in_offset=bass.IndirectOffsetOnAxis(
                            ap=src_sb[:, t : t + 1], axis=0
                        ),
                    )
                # v: [128, TT*8] zero-padded so pass A fills rows 0-3 of the
                # scatter psum and pass B rows 4-7.
                v = const.tile([128, TT * 8], f32, tag=f"v{pname}")
                nc.vector.memset(v[:], 0.0)
                for t in range(TT):
                    nc.vector.tensor_scalar_mul(
                        v[:, t * 8 + col0 : t * 8 + col0 + D],
                        gath[:, t * D : (t + 1) * D],
                        ew_sb[:, t : t + 1],
                    )
                passes.append((pos_sb, v))

            # ---- scatter: psum[8, SLAB] += v.T @ onehot per slab ----
            for s in range(SLABS):
                t_lo, t_hi = (0, T0) if s == 0 else (T0, TT)
                mpsum = psS.tile([2 * D, SLAB], f32, tag="scat")
                first = True
                for pi, (pos_sb, v) in enumerate(passes):
                    for t in range(t_lo, t_hi):
                        oh = ohpool.tile([128, SLAB], f32, tag="oh")
                        nc.vector.tensor_tensor(
                            out=oh[:],
                            in0=pos_sb[:, t : t + 1].to_broadcast([128, SLAB]),
                            in1=iota_f[:],
                            op=mybir.AluOpType.is_equal,
                        )
                        nc.tensor.matmul(
                            mpsum[:],
                            lhsT=v[:, t * 8 : (t + 1) * 8],
                            rhs=oh[:],
                            start=first,
                            stop=(pi == 1 and t == t_hi - 1),
                        )
                        first = False
                nc.vector.tensor_copy(
                    MT_sb[0 : 2 * D, s * SLAB : (s + 1) * SLAB], mpsum[:]
                )

            # ---- MLP: y = sigmoid(tanh(M @ W1 + b1) @ W2 + b2) ----
            for s in range(SLABS):
                hpsum = psH.tile([H, SLAB], f32, tag="h_ps")
                nc.tensor.matmul(
                    hpsum[:],
                    lhsT=W1_sb[:],
                    rhs=MT_sb[:, s * SLAB : (s + 1) * SLAB],
                    start=True,
                    stop=True,
                )
                h_sb = mlp_pool.tile([H, SLAB], f32, tag="h_sb")
                nc.scalar.activation(
                    h_sb[:], hpsum[:], mybir.ActivationFunctionType.Tanh,
                    bias=b1_sb[:],
                )
                ypsum = psY.tile([1, SLAB], f32, tag="y_ps")
                nc.tensor.matmul(
                    ypsum[:], lhsT=W2_sb[:], rhs=h_sb[:], start=True, stop=True
                )
                y_sb = mlp_pool.tile([1, SLAB], f32, tag="y_sb")
                nc.scalar.activation(
                    y_sb[:], ypsum[:], mybir.ActivationFunctionType.Sigmoid,
                    bias=b2_sb[:],
                )
                nc.sync.dma_start(
                    out=y[:, s * SLAB : (s + 1) * SLAB], in_=y_sb[:]
                )

    nc.compile()
    return nc


def _get_sparse_nc(T0: int, T1: int):
    key = (T0, T1)
    if key not in _cached_sparse:
        _cached_sparse[key] = _build_sparse(T0, T1)
    return _cached_sparse[key]


def _extract_onehot(R: np.ndarray):
    """Return idx[E] with R[idx[k], k] == 1 if R is exactly one-hot per
    column (and zero elsewhere); None otherwise."""
    rows, cols = np.nonzero(R)
    if cols.size != E:
        return None
    cnt = np.bincount(cols, minlength=E)
    if cnt.max() != 1 or cnt.min() != 1:
        return None
    if not np.all(R[rows, cols] == 1.0):
        return None
    idx = np.empty(E, np.int64)
    idx[cols] = rows
    return idx


def _pack_pass(tgt: np.ndarray, src: np.ndarray, e: np.ndarray):
    """Sort edges by target node, shard by target-node range across cores,
    pad each 512-node slab's edge list to a multiple of 128 slots.

    Returns (T0, T1, per_core list of (src_i32, pos_f32, ew_f32), each
    [128, T0+T1])."""
    order = np.argsort(tgt, kind="stable")
    tgt_s = tgt[order]
    src_s = src[order]
    e_s = e[order]
    bounds = np.searchsorted(tgt_s, np.arange(0, N + 1, SLAB))
    cnts = np.diff(bounds).reshape(CORES, SLABS)
    T0 = max(1, int(np.ceil(cnts[:, 0].max() / 128)))
    T1 = max(1, int(np.ceil(cnts[:, 1].max() / 128)))

    per_core = []
    for c in range(CORES):
        src_cols, pos_cols, ew_cols = [], [], []
        for s, Ts in ((0, T0), (1, T1)):
            g = c * SLABS + s
            lo, hi = bounds[g], bounds[g + 1]
            cnt = hi - lo
            pad = Ts * 128
            si = np.zeros(pad, np.int32)
            po = np.full(pad, -1.0, np.float32)
            ew = np.zeros(pad, np.float32)
            si[:cnt] = src_s[lo:hi]
            po[:cnt] = (tgt_s[lo:hi] - (c * NPC + s * SLAB)).astype(np.float32)
            ew[:cnt] = e_s[lo:hi]
            # slot j = t*128 + p  ->  column t, partition p
            src_cols.append(si.reshape(Ts, 128).T)
            pos_cols.append(po.reshape(Ts, 128).T)
            ew_cols.append(ew.reshape(Ts, 128).T)
        per_core.append(
            (
                np.ascontiguousarray(np.concatenate(src_cols, axis=1)),
                np.ascontiguousarray(np.concatenate(pos_cols, axis=1)),
                np.ascontiguousarray(np.concatenate(ew_cols, axis=1)),
            )
        )
    return T0, T1, per_core


def _prepare_sparse(X, e, idx_i, idx_o, W1, b1, W2, b2):
    X = np.ascontiguousarray(np.asarray(X, dtype=np.float32))
    e_flat = np.asarray(e, dtype=np.float32).reshape(-1)
    XT = np.ascontiguousarray(X.T)

    # pass A: mi[n] = sum_{k: idx_i[k]=n} e[k] * X[idx_o[k]]
    TA0, TA1, packA = _pack_pass(idx_i, idx_o, e_flat)
    # pass B: mo[n] = sum_{k: idx_o[k]=n} e[k] * X[idx_i[k]]
    TB0, TB1, packB = _pack_pass(idx_o, idx_i, e_flat)
    T0 = max(TA0, TB0)
    T1 = max(TA1, TB1)

    def _widen(arrs, Ts_have, fill):
        """Re-pad per-slab column blocks from (TA0, TA1) to (T0, T1)."""
        sa, pa, ea = arrs
        th0, th1 = Ts_have
        if th0 == T0 and th1 == T1:
            return arrs
        out = []
        for a, f in ((sa, 0), (pa, fill), (ea, 0)):
            blk0 = a[:, :th0]
            blk1 = a[:, th0:]
            w0 = np.full((128, T0), f, a.dtype)
            w0[:, :th0] = blk0
            w1 = np.full((128, T1), f, a.dtype)
            w1[:, :th1] = blk1
            out.append(np.ascontiguousarray(np.concatenate([w0, w1], axis=1)))
        return tuple(out)

    W1c = np.ascontiguousarray(np.asarray(W1, dtype=np.float32))
    b1c = np.ascontiguousarray(np.asarray(b1, dtype=np.float32).reshape(H, 1))
    W2c = np.ascontiguousarray(np.asarray(W2, dtype=np.float32).reshape(H, 1))
    b2c = np.ascontiguousarray(np.asarray(b2, dtype=np.float32).reshape(1, 1))

    in_maps = []
    for c in range(CORES):
        sa, pa, ea = _widen(packA[c], (TA0, TA1), -1.0)
        sb, pb, eb = _widen(packB[c], (TB0, TB1), -1.0)
        in_maps.append(
            {
                "Xtab": X,
                "XTl": np.ascontiguousarray(XT[:, c * NPC : (c + 1) * NPC]),
                "srcA": sa,
                "posA": pa,
                "ewA": ea,
                "srcB": sb,
                "posB": pb,
                "ewB": eb,
                "W1": W1c,
                "b1": b1c,
                "W2": W2c,
                "b2": b2c,
            }
        )
    return T0, T1, in_maps


def plan(inputs: dict):
    """Resolve the execution plan for these inputs.

    Returns (nc, in_maps, assemble) where assemble(results) -> full [N, 1]
    output. Falls back to the dense kernel when Ri/Ro are not one-hot."""
    Ri = np.asarray(inputs["Ri"], dtype=np.float32)
    Ro = np.asarray(inputs["Ro"], dtype=np.float32)
    idx_i = _extract_onehot(Ri)
    idx_o = _extract_onehot(Ro)
    if idx_i is not None and idx_o is not None:
        T0, T1, in_maps = _prepare_sparse(
            inputs["X"], inputs["e"], idx_i, idx_o,
            inputs["W1"], inputs["b1"], inputs["W2"], inputs["b2"],
        )
        nc = _get_sparse_nc(T0, T1)

        def assemble(results):
            return np.concatenate(
                [
                    np.asarray(results[c]["y"], dtype=np.float32).reshape(NPC)
                    for c in range(CORES)
                ]
            ).reshape(N, 1)

        return nc, in_maps, assemble

    # dense fallback
    in_maps = _prepare_in_maps(
        inputs["X"], inputs["e"], Ri, Ro,
        inputs["W1"], inputs["b1"], inputs["W2"], inputs["b2"],
    )
    nc = _get_nc()

    def assemble(results):
        return np.asarray(results[0]["y"], dtype=np.float32).reshape(N, 1)

    return nc, in_maps, assemble


def kernel(**inputs) -> np.ndarray:
    global _last_exec_time_ns
    from concourse import bass_utils

    nc, in_maps, assemble = plan(inputs)
    trace = os.environ.get("KERNEL_TRACE", "") == "1"
    res = bass_utils.run_bass_kernel_spmd(
        nc, in_maps, core_ids=list(range(CORES)), trace=trace
    )
    _last_exec_time_ns = res.exec_time_ns
    return assemble(res.results)


# --------------------------------------------------------------------------
# dense fallback path (original kernel, used only if Ri/Ro aren't one-hot)
# --------------------------------------------------------------------------

def _build(collective: bool = True, phases: int = 4, r_dtype: str = "float16"):
    # phases: 1=gather only, 2=+scatter, 3=+allreduce, 4=+mlp (full)
    import concourse.bass as bass
    import concourse.bacc as bacc
    import concourse.mybir as mybir
    import concourse.tile as tile

    f32 = mybir.dt.float32
    f16 = mybir.dt.float16
    fR = getattr(mybir.dt, r_dtype)

    nc = bacc.Bacc(
        "TRN2",
        target_bir_lowering=False,
        debug=False,
        num_devices=CORES if collective else 1,
    )

    Ri_nat = nc.dram_tensor("Ri_nat", [N, ESH], fR, kind="ExternalInput").ap()
    Ro_nat = nc.dram_tensor("Ro_nat", [N, ESH], fR, kind="ExternalInput").ap()
    RiT = nc.dram_tensor("RiT", [ESH, N], fR, kind="ExternalInput").ap()
    RoT = nc.dram_tensor("RoT", [ESH, N], fR, kind="ExternalInput").ap()
    Xg = nc.dram_tensor("Xg", [128, NCH * D], f16, kind="ExternalInput").ap()
    XT = nc.dram_tensor("XT", [D, N], f32, kind="ExternalInput").ap()
    esh = nc.dram_tensor("esh", [128, ECH], f32, kind="ExternalInput").ap()
    W1 = nc.dram_tensor("W1", [3 * D, H], f32, kind="ExternalInput").ap()
    b1 = nc.dram_tensor("b1", [H, 1], f32, kind="ExternalInput").ap()
    W2 = nc.dram_tensor("W2", [H, 1], f32, kind="ExternalInput").ap()
    b2 = nc.dram_tensor("b2", [1, 1], f32, kind="ExternalInput").ap()
    y = nc.dram_tensor("y", [1, N], f32, kind="ExternalOutput").ap()

    with tile.TileContext(nc) as tc:
        with (
            tc.tile_pool(name="const", bufs=1) as const,
            tc.tile_pool(name="gslab", bufs=3) as gslab_pool,
            tc.tile_pool(name="sslab", bufs=2) as sslab_pool,
            tc.tile_pool(name="small", bufs=1) as small,
            tc.tile_pool(name="mlp", bufs=2) as mlp_pool,
            tc.tile_pool(name="psA", bufs=2, space="PSUM") as psA,
            tc.tile_pool(name="psB", bufs=2, space="PSUM") as psB,
            tc.tile_pool(name="dram", bufs=1, space="DRAM") as dram,
        ):
            Xg_sb = const.tile([128, NCH * D], f16)
            nc.sync.dma_start(out=Xg_sb[:], in_=Xg[:])
            e_sb = const.tile([128, ECH], f32)
            nc.sync.dma_start(out=e_sb[:], in_=esh[:])
            W1_sb = const.tile([3 * D, H], f32)
            nc.sync.dma_start(out=W1_sb[:], in_=W1[:])
            b1_sb = const.tile([H, 1], f32)
            nc.sync.dma_start(out=b1_sb[:], in_=b1[:])
            W2_sb = const.tile([H, 1], f32)
            nc.sync.dma_start(out=W2_sb[:], in_=W2[:])
            b2_sb = const.tile([1, 1], f32)
            nc.sync.dma_start(out=b2_sb[:], in_=b2[:])

            MT_sb = small.tile([3 * D, N], f32)
            nc.sync.dma_start(out=MT_sb[2 * D : 3 * D, :], in_=XT[:])

            bvi = small.tile([128, ECH * 12], f16)
            bvo = small.tile([128, ECH * 12], f16)
            nc.vector.memset(bvi[:], 0.0)
            nc.vector.memset(bvo[:], 0.0)

            for Rnat, dst, col0, acc_tag in (
                (Ri_nat, bvo, 4, "bacc_i"),
                (Ro_nat, bvi, 0, "bacc_o"),
            ):
                bacc_t = small.tile([128, ECH * D], f32, tag=acc_tag)
                for nch in range(NCH):
                    slab = gslab_pool.tile([128, ESH], fR, tag="gs")
                    nc.sync.dma_start(
                        out=slab[:], in_=Rnat[nch * 128 : (nch + 1) * 128, :]
                    )
                    bpsum = psA.tile([128, ECH * D], f32, tag="gather_ps")
                    for ech in range(ECH):
                        nc.tensor.matmul(
                            bpsum[:, ech * D : (ech + 1) * D],
                            lhsT=slab[:, ech * 128 : (ech + 1) * 128],
                            rhs=Xg_sb[:, nch * D : (nch + 1) * D],
                            start=True,
                            stop=True,
                        )
                    if nch == 0:
                        nc.vector.tensor_copy(bacc_t[:], bpsum[:])
                    else:
                        nc.vector.tensor_add(bacc_t[:], bacc_t[:], bpsum[:])
                for ech in range(ECH):
                    nc.vector.tensor_scalar_mul(
                        dst[:, ech * 12 + col0 : ech * 12 + col0 + D],
                        bacc_t[:, ech * D : (ech + 1) * D],
                        e_sb[:, ech : ech + 1],
                    )

            RiT3 = RiT.rearrange("(ec p) n -> p ec n", p=128)
            RoT3 = RoT.rearrange("(ec p) n -> p ec n", p=128)
            for ns in range(NS if phases >= 2 else 0):
                mpsum = psB.tile([3 * D, NSLAB], f32, tag="scat_ps")
                first = True
                for RT3, bv, stag in ((RiT3, bvi, "ssi"), (RoT3, bvo, "sso")):
                    tslab = sslab_pool.tile([128, ECH, NSLAB], fR, tag=stag)
                    nc.sync.dma_start(
                        out=tslab[:],
                        in_=RT3[:, :, ns * NSLAB : (ns + 1) * NSLAB],
                    )
                    for ech in range(ECH):
                        nc.tensor.matmul(
                            mpsum[:],
                            lhsT=bv[:, ech * 12 : (ech + 1) * 12],
                            rhs=tslab[:, ech, :],
                            start=first,
                            stop=(bv is bvo and ech == ECH - 1),
                        )
                        first = False
                nc.vector.tensor_copy(
                    MT_sb[0 : 2 * D, ns * NSLAB : (ns + 1) * NSLAB],
                    mpsum[0 : 2 * D, :],
                )

            if collective and phases >= 3:
                ar_in = dram.tile([2 * D, N], f32)
                ar_out = dram.tile([2 * D, N], f32, addr_space="Shared")
                nc.gpsimd.dma_start(out=ar_in[:], in_=MT_sb[0 : 2 * D, :])
                nc.gpsimd.collective_compute(
                    "AllReduce",
                    mybir.AluOpType.add,
                    replica_groups=[list(range(CORES))],
                    ins=[ar_in.opt()],
                    outs=[ar_out.opt()],
                )
                nc.gpsimd.dma_start(out=MT_sb[0 : 2 * D, :], in_=ar_out[:])

            for ns in range(NS if phases >= 4 else 0):
                hpsum = psB.tile([H, NSLAB], f32, tag="h_ps")
                nc.tensor.matmul(
                    hpsum[:],
                    lhsT=W1_sb[:],
                    rhs=MT_sb[:, ns * NSLAB : (ns + 1) * NSLAB],
                    start=True,
                    stop=True,
                )
                h_sb = mlp_pool.tile([H, NSLAB], f32, tag="h_sb")
                nc.scalar.activation(
                    h_sb[:], hpsum[:], mybir.ActivationFunctionType.Tanh,
                    bias=b1_sb[:],
                )
                ypsum = psB.tile([1, NSLAB], f32, tag="y_ps")
                nc.tensor.matmul(
                    ypsum[:], lhsT=W2_sb[:], rhs=h_sb[:], start=True, stop=True
                )
                y_sb = mlp_pool.tile([1, NSLAB], f32, tag="y_sb")
                nc.scalar.activation(
                    y_sb[:], ypsum[:], mybir.ActivationFunctionType.Sigmoid,
                    bias=b2_sb[:],
                )
                nc.sync.dma_start(
                    out=y[:, ns * NSLAB : (ns + 1) * NSLAB], in_=y_sb[:]
                )

    nc.compile()
    return nc


def _get_nc(r_dtype: str = "float16"):
    if r_dtype not in _cached:
        _cached[r_dtype] = _build(r_dtype=r_dtype)
    return _cached[r_dtype]


def _prepare_in_maps(X, e, Ri, Ro, W1, b1, W2, b2, r_dtype: str = "float16"):
    X = np.asarray(X, dtype=np.float32)
    e = np.asarray(e, dtype=np.float32)
    W1 = np.asarray(W1, dtype=np.float32)
    b1 = np.asarray(b1, dtype=np.float32)
    W2 = np.asarray(W2, dtype=np.float32)
    b2 = np.asarray(b2, dtype=np.float32)

    rdt = np.float16
    Ri16 = np.asarray(Ri, dtype=np.float32).astype(rdt)
    Ro16 = np.asarray(Ro, dtype=np.float32).astype(rdt)
    RiT16 = np.ascontiguousarray(Ri16.T)
    RoT16 = np.ascontiguousarray(Ro16.T)

    X16 = X.astype(np.float16)
    Xg = np.ascontiguousarray(
        X16.reshape(NCH, 128, D).transpose(1, 0, 2).reshape(128, NCH * D)
    )
    XT = np.ascontiguousarray(X.T)

    b1c = np.ascontiguousarray(b1.reshape(H, 1))
    b2c = np.ascontiguousarray(b2.reshape(1, 1))
    W1c = np.ascontiguousarray(W1)
    W2c = np.ascontiguousarray(W2.reshape(H, 1))

    in_maps = []
    for c in range(CORES):
        sh = slice(c * ESH, (c + 1) * ESH)
        e_c = np.ascontiguousarray(
            e.reshape(-1)[sh].reshape(ECH, 128).T
        ).astype(np.float32)
        in_maps.append(
            {
                "Ri_nat": np.ascontiguousarray(Ri16[:, sh]),
                "Ro_nat": np.ascontiguousarray(Ro16[:, sh]),
                "RiT": RiT16[sh],
                "RoT": RoT16[sh],
                "Xg": Xg,
                "XT": XT,
                "esh": e_c,
                "W1": W1c,
                "b1": b1c,
                "W2": W2c,
                "b2": b2c,
            }
        )
    return in_maps


# revision 21
# speedup vs baseline: 3.6046x; 1.0913x over previous
"""Trainium2 Bass kernel for nn_NodeNet (GNN message passing).

Reference computation:
    bo = Ro.T @ X            [E, D]   (gather per-edge source feats)
    bi = Ri.T @ X            [E, D]
    mi = (Ri * e.T) @ bo     [N, D]   (edge-weighted scatter-add)
    mo = (Ro * e.T) @ bi     [N, D]
    M  = [mi, mo, X]         [N, 3D]
    y  = sigmoid(tanh(M @ W1 + b1) @ W2 + b2)

Fast path (sparse, 8 NeuronCores, node-range sharded):
  Ri/Ro are one-hot incidence matrices, so each edge k is fully described
  by (idx_i[k], idx_o[k], e[k]).  Shipping the dense [N, E] matrices to
  the device costs ~1.6 GB per execution; shipping indices costs ~3 MB.

  - Host extracts idx_i/idx_o (exact-one-hot validated; dense fallback
    otherwise) and shards EDGES BY TARGET-NODE RANGE: core c owns nodes
    [c*1024, (c+1)*1024) and receives the edges whose target (idx_i for
    mi, idx_o for mo) lands in its range.  Partial sums never cross
    cores, so NO collective is needed.
  - On device, per pass (mi / mo):
      1. one indirect DMA gathers X[src] rows for all edges ([128, T*4]),
      2. DVE scales rows by e,
      3. per 512-node slab, DVE builds one-hot [128, 512] scatter tiles
         (is_equal against an iota) and the PE accumulates
         psum[8, 512] += v[128, 8].T @ onehot  (mi in rows 0-3, mo in
         rows 4-7 via zero-padded lhsT columns).
  - The tiny MLP runs per slab exactly as in the dense kernel.
"""

import os
import numpy as np

N = 8192
E = 24576
D = 4
H = 100
CORES = 8
NPC = N // CORES          # 1024 nodes per core
SLAB = 512                # psum-width node slab
SLABS = NPC // SLAB       # 2 slabs per core

# dense-fallback constants
ESH = E // CORES
NCH = N // 128
ECH = ESH // 128
NSLAB = 512
NS = N // NSLAB

_last_exec_time_ns = None
_cached = {}
_cached_sparse = {}


# --------------------------------------------------------------------------
# sparse fast path
# --------------------------------------------------------------------------

def _build_sparse(T0: int, T1: int):
    import concourse.bass as bass
    import concourse.bacc as bacc
    import concourse.mybir as mybir
    import concourse.tile as tile

    f32 = mybir.dt.float32
    f16 = mybir.dt.float16
    i32 = mybir.dt.int32
    TT = T0 + T1

    nc = bacc.Bacc(
        "TRN2",
        target_bir_lowering=False,
        debug=False,
        num_devices=CORES,
        num_swdge_queues=2,
    )

    # X rows padded to 64 f32 (256 B) — dma_gather's minimum element size.
    GE = 64
    Xpad = nc.dram_tensor("Xpad", [N, GE], f32, kind="ExternalInput").ap()
    XTl = nc.dram_tensor("XTl", [D, NPC], f32, kind="ExternalInput").ap()
    # int16 indices in dma_gather's wrapped layout: idx j at partition
    # j % 16, column j // 16 (replicated across the 8 gpsimd cores).
    idxA = nc.dram_tensor("idxA", [128, TT * 8], mybir.dt.int16,
                          kind="ExternalInput").ap()
    posA = nc.dram_tensor("posA", [128, TT], f32, kind="ExternalInput").ap()
    ewA = nc.dram_tensor("ewA", [128, TT], f32, kind="ExternalInput").ap()
    idxB = nc.dram_tensor("idxB", [128, TT * 8], mybir.dt.int16,
                          kind="ExternalInput").ap()
    posB = nc.dram_tensor("posB", [128, TT], f32, kind="ExternalInput").ap()
    ewB = nc.dram_tensor("ewB", [128, TT], f32, kind="ExternalInput").ap()
    W1 = nc.dram_tensor("W1", [3 * D, H], f32, kind="ExternalInput").ap()
    b1 = nc.dram_tensor("b1", [H, 1], f32, kind="ExternalInput").ap()
    W2 = nc.dram_tensor("W2", [H, 1], f32, kind="ExternalInput").ap()
    b2 = nc.dram_tensor("b2", [1, 1], f32, kind="ExternalInput").ap()
    y = nc.dram_tensor("y", [1, NPC], f32, kind="ExternalOutput").ap()

    with tile.TileContext(nc) as tc:
        with (
            tc.tile_pool(name="const", bufs=1) as const,
            tc.tile_pool(name="oh", bufs=4) as ohpool,
            tc.tile_pool(name="mlp", bufs=2) as mlp_pool,
            tc.tile_pool(name="psS", bufs=2, space="PSUM") as psS,
            tc.tile_pool(name="psH", bufs=2, space="PSUM") as psH,
            tc.tile_pool(name="psY", bufs=2, space="PSUM") as psY,
        ):
            # ---- resident small tensors ----
            W1_sb = const.tile([3 * D, H], f32)
            nc.sync.dma_start(out=W1_sb[:], in_=W1[:])
            b1_sb = const.tile([H, 1], f32)
            nc.sync.dma_start(out=b1_sb[:], in_=b1[:])
            W2_sb = const.tile([H, 1], f32)
            nc.sync.dma_start(out=W2_sb[:], in_=W2[:])
            b2_sb = const.tile([1, 1], f32)
            nc.sync.dma_start(out=b2_sb[:], in_=b2[:])

            # M.T rows: 0-3 mi, 4-7 mo, 8-11 X
            MT_sb = const.tile([3 * D, NPC], f32)
            nc.sync.dma_start(out=MT_sb[2 * D : 3 * D, :], in_=XTl[:])

            iota_i = const.tile([128, SLAB], i32)
            nc.gpsimd.iota(
                iota_i[:], pattern=[[1, SLAB]], base=0, channel_multiplier=0
            )
            # fp16 is exact for 0..511 and the scatter one-hots, and runs the
            # DVE compares and PE matmuls at full rate (fp32 is 4 cyc/row).
            iota_f = const.tile([128, SLAB], f16)
            nc.vector.tensor_copy(iota_f[:], iota_i[:])

            # ---- per pass: gather + e-scale ----
            passes = []
            for qnum, (pname, idxT, posT, ewT, col0) in enumerate((
                ("A", idxA, posA, ewA, 0),
                ("B", idxB, posB, ewB, D),
            )):
                idx_sb = const.tile([128, TT * 8], mybir.dt.int16,
                                    tag=f"idx{pname}")
                nc.sync.dma_start(out=idx_sb[:], in_=idxT[:])
                pos_sb = const.tile([128, TT], f32, tag=f"pos{pname}")
                nc.sync.dma_start(out=pos_sb[:], in_=posT[:])
                ew_sb = const.tile([128, TT], f32, tag=f"ew{pname}")
                nc.sync.dma_start(out=ew_sb[:], in_=ewT[:])

                # dma_gather: slot j -> partition j % 128, column j // 128
                # (matches the host packing layout). HW rejects >~1024
                # indices per call, so chunk in groups of 8 tiles.
                gath = const.tile([128, TT * GE], f32, tag=f"gath{pname}")
                gath3 = gath[:].rearrange("p (t c) -> p t c", c=GE)
                for t0 in range(0, TT, 8):
                    ct = min(8, TT - t0)
                    nc.gpsimd.dma_gather(
                        out_ap=gath3[:, t0 : t0 + ct, :],
                        in_ap=Xpad[:],
                        idxs_ap=idx_sb[:, t0 * 8 : (t0 + ct) * 8],
                        num_idxs=ct * 128,
                        num_idxs_reg=ct * 128,
                        elem_size=GE,
                        queue_num=qnum,
                    )
                # v: [128, TT*8] zero-padded so pass A fills rows 0-3 of the
                # scatter psum and pass B rows 4-7.
                v = const.tile([128, TT * 8], f16, tag=f"v{pname}")
                nc.vector.memset(v[:], 0.0)
                nc.vector.tensor_tensor(
                    out=v[:].rearrange("p (t c) -> p t c", c=8)[
                        :, :, col0 : col0 + D
                    ],
                    in0=gath[:].rearrange("p (t c) -> p t c", c=GE)[
                        :, :, 0:D
                    ],
                    in1=ew_sb[:, :, None].to_broadcast([128, TT, D]),
                    op=mybir.AluOpType.mult,
                )
                passes.append((pos_sb, v))

            # ---- scatter: psum[8, SLAB] += v.T @ onehot per slab ----
            for s in range(SLABS):
                t_lo, t_hi = (0, T0) if s == 0 else (T0, TT)
                mpsum = psS.tile([2 * D, SLAB], f32, tag="scat")
                first = True
                for pi, (pos_sb, v) in enumerate(passes):
                    for t in range(t_lo, t_hi):
                        # tensor_scalar keeps every non-scalar operand packed
                        # fp16, enabling the DVE 2x/4x fast modes.
                        oh = ohpool.tile([128, SLAB], f16, tag="oh")
                        nc.vector.tensor_scalar(
                            oh[:],
                            iota_f[:],
                            pos_sb[:, t : t + 1],
                            None,
                            op0=mybir.AluOpType.is_equal,
                        )
                        nc.tensor.matmul(
                            mpsum[:],
                            lhsT=v[:, t * 8 : (t + 1) * 8],
                            rhs=oh[:],
                            start=first,
                            stop=(pi == 1 and t == t_hi - 1),
                        )
                        first = False
                nc.vector.tensor_copy(
                    MT_sb[0 : 2 * D, s * SLAB : (s + 1) * SLAB], mpsum[:]
                )

            # ---- MLP: y = sigmoid(tanh(M @ W1 + b1) @ W2 + b2) ----
            for s in range(SLABS):
                hpsum = psH.tile([H, SLAB], f32, tag="h_ps")
                nc.tensor.matmul(
                    hpsum[:],
                    lhsT=W1_sb[:],
                    rhs=MT_sb[:, s * SLAB : (s + 1) * SLAB],
                    start=True,
                    stop=True,
                )
                h_sb = mlp_pool.tile([H, SLAB], f32, tag="h_sb")
                nc.scalar.activation(
                    h_sb[:], hpsum[:], mybir.ActivationFunctionType.Tanh,
                    bias=b1_sb[:],
                )
                ypsum = psY.tile([1, SLAB], f32, tag="y_ps")
                nc.tensor.matmul(
                    ypsum[:], lhsT=W2_sb[:], rhs=h_sb[:], start=True, stop=True
                )
                y_sb = mlp_pool.tile([1, SLAB], f32, tag="y_sb")
                nc.scalar.activation(
                    y_sb[:], ypsum[:], mybir.ActivationFunctionType.Sigmoid,
                    bias=b2_sb[:],
                )
                nc.sync.dma_start(
                    out=y[:, s * SLAB : (s + 1) * SLAB], in_=y_sb[:]
                )

    nc.compile()
    return nc


def _get_sparse_nc(T0: int, T1: int):
    key = (T0, T1)
    if key not in _cached_sparse:
        _cached_sparse[key] = _build_sparse(T0, T1)
    return _cached_sparse[key]


def _extract_onehot(R: np.ndarray):
    """Return idx[E] with R[idx[k], k] == 1 if R is exactly one-hot per
    column (and zero elsewhere); None otherwise."""
    rows, cols = np.nonzero(R)
    if cols.size != E:
        return None
    cnt = np.bincount(cols, minlength=E)
    if cnt.max() != 1 or cnt.min() != 1:
        return None
    if not np.all(R[rows, cols] == 1.0):
        return None
    idx = np.empty(E, np.int64)
    idx[cols] = rows
    return idx


def _pass_counts(tgt: np.ndarray):
    order = np.argsort(tgt, kind="stable")
    bounds = np.searchsorted(tgt[order], np.arange(0, N + 1, SLAB))
    cnts = np.diff(bounds).reshape(CORES, SLABS)
    return order, bounds, cnts


def _pack_pass(tgt, src, e, order, bounds, T0, T1):
    """Sort edges by target node, shard by target-node range across cores,
    pad each 512-node slab's edge list to T0/T1 128-slot tiles.

    Returns per_core list of (idx16, pos_f32, ew_f32)."""
    tgt_s = tgt[order]
    src_s = src[order]
    e_s = e[order]

    per_core = []
    for c in range(CORES):
        src_cols, pos_cols, ew_cols = [], [], []
        for s, Ts in ((0, T0), (1, T1)):
            g = c * SLABS + s
            lo, hi = bounds[g], bounds[g + 1]
            cnt = hi - lo
            pad = Ts * 128
            si = np.zeros(pad, np.int16)
            po = np.full(pad, -1.0, np.float32)
            ew = np.zeros(pad, np.float32)
            si[:cnt] = src_s[lo:hi]
            po[:cnt] = (tgt_s[lo:hi] - (c * NPC + s * SLAB)).astype(np.float32)
            ew[:cnt] = e_s[lo:hi]
            # slot j = t*128 + p  ->  column t, partition p
            src_cols.append(si)
            pos_cols.append(po.reshape(Ts, 128).T)
            ew_cols.append(ew.reshape(Ts, 128).T)
        # dma_gather wrapped index layout: idx j at partition j % 16,
        # column j // 16, replicated across the 8 gpsimd cores.
        flat = np.concatenate(src_cols)
        idx16 = np.tile(flat.reshape(-1, 16).T, (8, 1))
        per_core.append(
            (
                np.ascontiguousarray(idx16),
                np.ascontiguousarray(np.concatenate(pos_cols, axis=1)),
                np.ascontiguousarray(np.concatenate(ew_cols, axis=1)),
            )
        )
    return per_core


def _prepare_sparse(X, e, idx_i, idx_o, W1, b1, W2, b2):
    X = np.ascontiguousarray(np.asarray(X, dtype=np.float32))
    e_flat = np.asarray(e, dtype=np.float32).reshape(-1)
    XT = np.ascontiguousarray(X.T)
    Xpad = np.zeros((N, 64), np.float32)
    Xpad[:, :D] = X

    # pass A: mi[n] = sum_{k: idx_i[k]=n} e[k] * X[idx_o[k]]
    ordA, bndA, cntA = _pass_counts(idx_i)
    # pass B: mo[n] = sum_{k: idx_o[k]=n} e[k] * X[idx_i[k]]
    ordB, bndB, cntB = _pass_counts(idx_o)
    T0 = max(1, int(np.ceil(max(cntA[:, 0].max(), cntB[:, 0].max()) / 128)))
    T1 = max(1, int(np.ceil(max(cntA[:, 1].max(), cntB[:, 1].max()) / 128)))
    packA = _pack_pass(idx_i, idx_o, e_flat, ordA, bndA, T0, T1)
    packB = _pack_pass(idx_o, idx_i, e_flat, ordB, bndB, T0, T1)

    W1c = np.ascontiguousarray(np.asarray(W1, dtype=np.float32))
    b1c = np.ascontiguousarray(np.asarray(b1, dtype=np.float32).reshape(H, 1))
    W2c = np.ascontiguousarray(np.asarray(W2, dtype=np.float32).reshape(H, 1))
    b2c = np.ascontiguousarray(np.asarray(b2, dtype=np.float32).reshape(1, 1))

    in_maps = []
    for c in range(CORES):
        ia, pa, ea = packA[c]
        ib, pb, eb = packB[c]
        in_maps.append(
            {
                "Xpad": Xpad,
                "XTl": np.ascontiguousarray(XT[:, c * NPC : (c + 1) * NPC]),
                "idxA": ia,
                "posA": pa,
                "ewA": ea,
                "idxB": ib,
                "posB": pb,
                "ewB": eb,
                "W1": W1c,
                "b1": b1c,
                "W2": W2c,
                "b2": b2c,
            }
        )
    return T0, T1, in_maps


def plan(inputs: dict):
    """Resolve the execution plan for these inputs.

    Returns (nc, in_maps, assemble) where assemble(results) -> full [N, 1]
    output. Falls back to the dense kernel when Ri/Ro are not one-hot."""
    Ri = np.asarray(inputs["Ri"], dtype=np.float32)
    Ro = np.asarray(inputs["Ro"], dtype=np.float32)
    idx_i = _extract_onehot(Ri)
    idx_o = _extract_onehot(Ro)
    if idx_i is not None and idx_o is not None:
        T0, T1, in_maps = _prepare_sparse(
            inputs["X"], inputs["e"], idx_i, idx_o,
            inputs["W1"], inputs["b1"], inputs["W2"], inputs["b2"],
        )
        nc = _get_sparse_nc(T0, T1)

        def assemble(results):
            return np.concatenate(
                [
                    np.asarray(results[c]["y"], dtype=np.float32).reshape(NPC)
                    for c in range(CORES)
                ]
            ).reshape(N, 1)

        return nc, in_maps, assemble

    # dense fallback
    in_maps = _prepare_in_maps(
        inputs["X"], inputs["e"], Ri, Ro,
        inputs["W1"], inputs["b1"], inputs["W2"], inputs["b2"],
    )
    nc = _get_nc()

    def assemble(results):
        return np.asarray(results[0]["y"], dtype=np.float32).reshape(N, 1)

    return nc, in_maps, assemble


def kernel(**inputs) -> np.ndarray:
    global _last_exec_time_ns
    from concourse import bass_utils

    nc, in_maps, assemble = plan(inputs)
    trace = os.environ.get("KERNEL_TRACE", "") == "1"
    res = bass_utils.run_bass_kernel_spmd(
        nc, in_maps, core_ids=list(range(CORES)), trace=trace
    )
    _last_exec_time_ns = res.exec_time_ns
    return assemble(res.results)


# --------------------------------------------------------------------------
# dense fallback path (original kernel, used only if Ri/Ro aren't one-hot)
# --------------------------------------------------------------------------

def _build(collective: bool = True, phases: int = 4, r_dtype: str = "float16"):
    # phases: 1=gather only, 2=+scatter, 3=+allreduce, 4=+mlp (full)
    import concourse.bass as bass
    import concourse.bacc as bacc
    import concourse.mybir as mybir
    import concourse.tile as tile

    f32 = mybir.dt.float32
    f16 = mybir.dt.float16
    fR = getattr(mybir.dt, r_dtype)

    nc = bacc.Bacc(
        "TRN2",
        target_bir_lowering=False,
        debug=False,
        num_devices=CORES if collective else 1,
    )

    Ri_nat = nc.dram_tensor("Ri_nat", [N, ESH], fR, kind="ExternalInput").ap()
    Ro_nat = nc.dram_tensor("Ro_nat", [N, ESH], fR, kind="ExternalInput").ap()
    RiT = nc.dram_tensor("RiT", [ESH, N], fR, kind="ExternalInput").ap()
    RoT = nc.dram_tensor("RoT", [ESH, N], fR, kind="ExternalInput").ap()
    Xg = nc.dram_tensor("Xg", [128, NCH * D], f16, kind="ExternalInput").ap()
    XT = nc.dram_tensor("XT", [D, N], f32, kind="ExternalInput").ap()
    esh = nc.dram_tensor("esh", [128, ECH], f32, kind="ExternalInput").ap()
    W1 = nc.dram_tensor("W1", [3 * D, H], f32, kind="ExternalInput").ap()
    b1 = nc.dram_tensor("b1", [H, 1], f32, kind="ExternalInput").ap()
    W2 = nc.dram_tensor("W2", [H, 1], f32, kind="ExternalInput").ap()
    b2 = nc.dram_tensor("b2", [1, 1], f32, kind="ExternalInput").ap()
    y = nc.dram_tensor("y", [1, N], f32, kind="ExternalOutput").ap()

    with tile.TileContext(nc) as tc:
        with (
            tc.tile_pool(name="const", bufs=1) as const,
            tc.tile_pool(name="gslab", bufs=3) as gslab_pool,
            tc.tile_pool(name="sslab", bufs=2) as sslab_pool,
            tc.tile_pool(name="small", bufs=1) as small,
            tc.tile_pool(name="mlp", bufs=2) as mlp_pool,
            tc.tile_pool(name="psA", bufs=2, space="PSUM") as psA,
            tc.tile_pool(name="psB", bufs=2, space="PSUM") as psB,
            tc.tile_pool(name="dram", bufs=1, space="DRAM") as dram,
        ):
            Xg_sb = const.tile([128, NCH * D], f16)
            nc.sync.dma_start(out=Xg_sb[:], in_=Xg[:])
            e_sb = const.tile([128, ECH], f32)
            nc.sync.dma_start(out=e_sb[:], in_=esh[:])
            W1_sb = const.tile([3 * D, H], f32)
            nc.sync.dma_start(out=W1_sb[:], in_=W1[:])
            b1_sb = const.tile([H, 1], f32)
            nc.sync.dma_start(out=b1_sb[:], in_=b1[:])
            W2_sb = const.tile([H, 1], f32)
            nc.sync.dma_start(out=W2_sb[:], in_=W2[:])
            b2_sb = const.tile([1, 1], f32)
            nc.sync.dma_start(out=b2_sb[:], in_=b2[:])

            MT_sb = small.tile([3 * D, N], f32)
            nc.sync.dma_start(out=MT_sb[2 * D : 3 * D, :], in_=XT[:])

            bvi = small.tile([128, ECH * 12], f16)
            bvo = small.tile([128, ECH * 12], f16)
            nc.vector.memset(bvi[:], 0.0)
            nc.vector.memset(bvo[:], 0.0)

            for Rnat, dst, col0, acc_tag in (
                (Ri_nat, bvo, 4, "bacc_i"),
                (Ro_nat, bvi, 0, "bacc_o"),
            ):
                bacc_t = small.tile([128, ECH * D], f32, tag=acc_tag)
                for nch in range(NCH):
                    slab = gslab_pool.tile([128, ESH], fR, tag="gs")
                    nc.sync.dma_start(
                        out=slab[:], in_=Rnat[nch * 128 : (nch + 1) * 128, :]
                    )
                    bpsum = psA.tile([128, ECH * D], f32, tag="gather_ps")
                    for ech in range(ECH):
                        nc.tensor.matmul(
                            bpsum[:, ech * D : (ech + 1) * D],
                            lhsT=slab[:, ech * 128 : (ech + 1) * 128],
                            rhs=Xg_sb[:, nch * D : (nch + 1) * D],
                            start=True,
                            stop=True,
                        )
                    if nch == 0:
                        nc.vector.tensor_copy(bacc_t[:], bpsum[:])
                    else:
                        nc.vector.tensor_add(bacc_t[:], bacc_t[:], bpsum[:])
                for ech in range(ECH):
                    nc.vector.tensor_scalar_mul(
                        dst[:, ech * 12 + col0 : ech * 12 + col0 + D],
                        bacc_t[:, ech * D : (ech + 1) * D],
                        e_sb[:, ech : ech + 1],
                    )

            RiT3 = RiT.rearrange("(ec p) n -> p ec n", p=128)
            RoT3 = RoT.rearrange("(ec p) n -> p ec n", p=128)
            for ns in range(NS if phases >= 2 else 0):
                mpsum = psB.tile([3 * D, NSLAB], f32, tag="scat_ps")
                first = True
                for RT3, bv, stag in ((RiT3, bvi, "ssi"), (RoT3, bvo, "sso")):
                    tslab = sslab_pool.tile([128, ECH, NSLAB], fR, tag=stag)
                    nc.sync.dma_start(
                        out=tslab[:],
                        in_=RT3[:, :, ns * NSLAB : (ns + 1) * NSLAB],
                    )
                    for ech in range(ECH):
                        nc.tensor.matmul(
                            mpsum[:],
                            lhsT=bv[:, ech * 12 : (ech + 1) * 12],
                            rhs=tslab[:, ech, :],
                            start=first,
                            stop=(bv is bvo and ech == ECH - 1),
                        )
                        first = False
                nc.vector.tensor_copy(
                    MT_sb[0 : 2 * D, ns * NSLAB : (ns + 1) * NSLAB],
                    mpsum[0 : 2 * D, :],
                )

            if collective and phases >= 3:
                ar_in = dram.tile([2 * D, N], f32)
                ar_out = dram.tile([2 * D, N], f32, addr_space="Shared")
                nc.gpsimd.dma_start(out=ar_in[:], in_=MT_sb[0 : 2 * D, :])
                nc.gpsimd.collective_compute(
                    "AllReduce",
                    mybir.AluOpType.add,
                    replica_groups=[list(range(CORES))],
                    ins=[ar_in.opt()],
                    outs=[ar_out.opt()],
                )
                nc.gpsimd.dma_start(out=MT_sb[0 : 2 * D, :], in_=ar_out[:])

            for ns in range(NS if phases >= 4 else 0):
                hpsum = psB.tile([H, NSLAB], f32, tag="h_ps")
                nc.tensor.matmul(
                    hpsum[:],
                    lhsT=W1_sb[:],
                    rhs=MT_sb[:, ns * NSLAB : (ns + 1) * NSLAB],
                    start=True,
                    stop=True,
                )
                h_sb = mlp_pool.tile([H, NSLAB], f32, tag="h_sb")
                nc.scalar.activation(
                    h_sb[:], hpsum[:], mybir.ActivationFunctionType.Tanh,
                    bias=b1_sb[:],
                )
                ypsum = psB.tile([1, NSLAB], f32, tag="y_ps")
                nc.tensor.matmul(
                    ypsum[:], lhsT=W2_sb[:], rhs=h_sb[:], start=True, stop=True
                )
                y_sb = mlp_pool.tile([1, NSLAB], f32, tag="y_sb")
                nc.scalar.activation(
                    y_sb[:], ypsum[:], mybir.ActivationFunctionType.Sigmoid,
                    bias=b2_sb[:],
                )
                nc.sync.dma_start(
                    out=y[:, ns * NSLAB : (ns + 1) * NSLAB], in_=y_sb[:]
                )

    nc.compile()
    return nc


def _get_nc(r_dtype: str = "float16"):
    if r_dtype not in _cached:
        _cached[r_dtype] = _build(r_dtype=r_dtype)
    return _cached[r_dtype]


def _prepare_in_maps(X, e, Ri, Ro, W1, b1, W2, b2, r_dtype: str = "float16"):
    X = np.asarray(X, dtype=np.float32)
    e = np.asarray(e, dtype=np.float32)
    W1 = np.asarray(W1, dtype=np.float32)
    b1 = np.asarray(b1, dtype=np.float32)
    W2 = np.asarray(W2, dtype=np.float32)
    b2 = np.asarray(b2, dtype=np.float32)

    rdt = np.float16
    Ri16 = np.asarray(Ri, dtype=np.float32).astype(rdt)
    Ro16 = np.asarray(Ro, dtype=np.float32).astype(rdt)
    RiT16 = np.ascontiguousarray(Ri16.T)
    RoT16 = np.ascontiguousarray(Ro16.T)

    X16 = X.astype(np.float16)
    Xg = np.ascontiguousarray(
        X16.reshape(NCH, 128, D).transpose(1, 0, 2).reshape(128, NCH * D)
    )
    XT = np.ascontiguousarray(X.T)

    b1c = np.ascontiguousarray(b1.reshape(H, 1))
    b2c = np.ascontiguousarray(b2.reshape(1, 1))
    W1c = np.ascontiguousarray(W1)
    W2c = np.ascontiguousarray(W2.reshape(H, 1))

    in_maps = []
    for c in range(CORES):
        sh = slice(c * ESH, (c + 1) * ESH)
        e_c = np.ascontiguousarray(
            e.reshape(-1)[sh].reshape(ECH, 128).T
        ).astype(np.float32)
        in_maps.append(
            {
                "Ri_nat": np.ascontiguousarray(Ri16[:, sh]),
                "Ro_nat": np.ascontiguousarray(Ro16[:, sh]),
                "RiT": RiT16[sh],
                "RoT": RoT16[sh],
                "Xg": Xg,
                "XT": XT,
                "esh": e_c,
                "W1": W1c,
                "b1": b1c,
                "W2": W2c,
                "b2": b2c,
            }
        )
    return in_maps


# revision 26
# speedup vs baseline: 3.9977x; 1.1091x over previous
"""Trainium2 Bass kernel for nn_NodeNet (GNN message passing).

Reference computation:
    bo = Ro.T @ X            [E, D]   (gather per-edge source feats)
    bi = Ri.T @ X            [E, D]
    mi = (Ri * e.T) @ bo     [N, D]   (edge-weighted scatter-add)
    mo = (Ro * e.T) @ bi     [N, D]
    M  = [mi, mo, X]         [N, 3D]
    y  = sigmoid(tanh(M @ W1 + b1) @ W2 + b2)

Fast path (sparse, 8 NeuronCores, node-range sharded):
  Ri/Ro are one-hot incidence matrices, so each edge k is fully described
  by (idx_i[k], idx_o[k], e[k]).  Shipping the dense [N, E] matrices to
  the device costs ~1.6 GB per execution; shipping indices costs ~3 MB.

  - Host extracts idx_i/idx_o (exact-one-hot validated; dense fallback
    otherwise) and shards EDGES BY TARGET-NODE RANGE: core c owns nodes
    [c*1024, (c+1)*1024) and receives the edges whose target (idx_i for
    mi, idx_o for mo) lands in its range.  Partial sums never cross
    cores, so NO collective is needed.
  - On device, per pass (mi / mo):
      1. one indirect DMA gathers X[src] rows for all edges ([128, T*4]),
      2. DVE scales rows by e,
      3. per 512-node slab, DVE builds one-hot [128, 512] scatter tiles
         (is_equal against an iota) and the PE accumulates
         psum[8, 512] += v[128, 8].T @ onehot  (mi in rows 0-3, mo in
         rows 4-7 via zero-padded lhsT columns).
  - The tiny MLP runs per slab exactly as in the dense kernel.
"""

import os
import numpy as np

N = 8192
E = 24576
D = 4
H = 100
CORES = 8
NPC = N // CORES          # 1024 nodes per core
SSL = 256                 # scatter slab width (psum tile)
NSSL = NPC // SSL         # 4 scatter slabs per core
MSL = 256                 # MLP slab width
NMSL = NPC // MSL         # 4 MLP slabs per core

# dense-fallback constants
ESH = E // CORES
NCH = N // 128
ECH = ESH // 128
NSLAB = 512
NS = N // NSLAB

_last_exec_time_ns = None
_cached = {}
_cached_sparse = {}


# --------------------------------------------------------------------------
# sparse fast path
# --------------------------------------------------------------------------

def _build_sparse(ts: tuple):
    import concourse.bass as bass
    import concourse.bacc as bacc
    import concourse.mybir as mybir
    import concourse.tile as tile

    f32 = mybir.dt.float32
    f16 = mybir.dt.float16
    i32 = mybir.dt.int32
    TT = sum(ts)

    nc = bacc.Bacc(
        "TRN2",
        target_bir_lowering=False,
        debug=False,
        num_devices=CORES,
        num_swdge_queues=4,
    )

    # X rows padded to 64 f32 (256 B) — dma_gather's minimum element size.
    GE = 64
    Xpad = nc.dram_tensor("Xpad", [N, GE], f32, kind="ExternalInput").ap()
    XTl = nc.dram_tensor("XTl", [D, NPC], f16, kind="ExternalInput").ap()
    # int16 indices in dma_gather's wrapped layout: idx j at partition
    # j % 16, column j // 16 (replicated across the 8 gpsimd cores).
    idxA = nc.dram_tensor("idxA", [128, TT * 8], mybir.dt.int16,
                          kind="ExternalInput").ap()
    # metaX: cols [0, TT) = pos, [TT, 2*TT) = ew
    metaA = nc.dram_tensor("metaA", [128, 2 * TT], f32,
                           kind="ExternalInput").ap()
    idxB = nc.dram_tensor("idxB", [128, TT * 8], mybir.dt.int16,
                          kind="ExternalInput").ap()
    metaB = nc.dram_tensor("metaB", [128, 2 * TT], f32,
                           kind="ExternalInput").ap()
    W1 = nc.dram_tensor("W1", [3 * D, H], f16, kind="ExternalInput").ap()
    b1 = nc.dram_tensor("b1", [H, 1], f32, kind="ExternalInput").ap()
    W2 = nc.dram_tensor("W2", [H, 1], f16, kind="ExternalInput").ap()
    b2 = nc.dram_tensor("b2", [1, 1], f32, kind="ExternalInput").ap()
    y = nc.dram_tensor("y", [1, NPC], f32, kind="ExternalOutput").ap()

    with tile.TileContext(nc) as tc:
        with (
            tc.tile_pool(name="const", bufs=1) as const,
            tc.tile_pool(name="oh", bufs=4) as ohpool,
            tc.tile_pool(name="mlp", bufs=2) as mlp_pool,
            tc.tile_pool(name="psS", bufs=2, space="PSUM") as psS,
            tc.tile_pool(name="psH", bufs=2, space="PSUM") as psH,
            tc.tile_pool(name="psY", bufs=2, space="PSUM") as psY,
        ):
            # ---- gather-critical inputs first: the SP DMA queue is
            # processed in program order, and the gathers wait on these ----
            idx_sbs, meta_sbs = {}, {}
            for pname, idxT, metaT in (("A", idxA, metaA), ("B", idxB, metaB)):
                idx_sb = const.tile([128, TT * 8], mybir.dt.int16,
                                    tag=f"idx{pname}")
                nc.sync.dma_start(out=idx_sb[:], in_=idxT[:])
                meta_sb = const.tile([128, 2 * TT], f32, tag=f"meta{pname}")
                nc.sync.dma_start(out=meta_sb[:], in_=metaT[:])
                idx_sbs[pname] = idx_sb
                meta_sbs[pname] = meta_sb

            # ---- resident small tensors ----
            W1_sb = const.tile([3 * D, H], f16)
            nc.sync.dma_start(out=W1_sb[:], in_=W1[:])
            b1_sb = const.tile([H, 1], f32)
            nc.sync.dma_start(out=b1_sb[:], in_=b1[:])
            W2_sb = const.tile([H, 1], f16)
            nc.sync.dma_start(out=W2_sb[:], in_=W2[:])
            b2_sb = const.tile([1, 1], f32)
            nc.sync.dma_start(out=b2_sb[:], in_=b2[:])

            # M.T rows: 0-3 mi, 4-7 mo, 8-11 X
            MT_sb = const.tile([3 * D, NPC], f16)
            nc.sync.dma_start(out=MT_sb[2 * D : 3 * D, :], in_=XTl[:])

            iota_i = const.tile([128, SSL], i32)
            nc.gpsimd.iota(
                iota_i[:], pattern=[[1, SSL]], base=0, channel_multiplier=0
            )
            # fp16 is exact for 0..SSL and the scatter one-hots, and runs the
            # DVE compares and PE matmuls at full rate (fp32 is 4 cyc/row).
            iota_f = const.tile([128, SSL], f16)
            nc.vector.tensor_copy(iota_f[:], iota_i[:])

            # ---- per pass: gather + e-scale ----
            passes = []
            for qnum, (pname, col0) in enumerate((("A", 0), ("B", D))):
                idx_sb = idx_sbs[pname]
                meta_sb = meta_sbs[pname]
                pos_sb = meta_sb[:, 0:TT]
                ew_sb = meta_sb[:, TT : 2 * TT]

                # dma_gather: slot j -> partition j % 128, column j // 128
                # (matches the host packing layout). HW rejects >~1024
                # indices per call, so chunk in groups of 8 tiles. Separate
                # tiles per chunk keep the dependency tracking chunk-local
                # (slice readers of one big tile would wait for ALL chunks),
                # and chunks alternate SWDGE queues for parallel desc-gen.
                vmap = {}
                for ci, t0 in enumerate(range(0, TT, 8)):
                    ct = min(8, TT - t0)
                    g = const.tile([128, ct * GE], f32,
                                   tag=f"gath{pname}{t0}")
                    g3 = g[:].rearrange("p (t c) -> p t c", c=GE)
                    nc.gpsimd.dma_gather(
                        out_ap=g3,
                        in_ap=Xpad[:],
                        idxs_ap=idx_sb[:, t0 * 8 : (t0 + ct) * 8],
                        num_idxs=ct * 128,
                        num_idxs_reg=ct * 128,
                        elem_size=GE,
                        queue_num=2 * qnum + (ci % 2),
                    )
                    # v: [128, ct*8] zero-padded so pass A fills rows 0-3 of
                    # the scatter psum and pass B rows 4-7.
                    v = const.tile([128, ct * 8], f16, tag=f"v{pname}{t0}")
                    nc.vector.memset(v[:], 0.0)
                    nc.vector.tensor_tensor(
                        out=v[:].rearrange("p (t c) -> p t c", c=8)[
                            :, :, col0 : col0 + D
                        ],
                        in0=g3[:, :, 0:D],
                        in1=ew_sb[:, t0 : t0 + ct][:, :, None].to_broadcast(
                            [128, ct, D]
                        ),
                        op=mybir.AluOpType.mult,
                    )
                    for t in range(t0, t0 + ct):
                        vmap[t] = (v, t - t0)
                passes.append((pos_sb, vmap))

            # ---- scatter: psum[8, SSL] += v.T @ onehot per slab ----
            t_bounds = [0]
            for t_s in ts:
                t_bounds.append(t_bounds[-1] + t_s)
            for s in range(NSSL):
                t_lo, t_hi = t_bounds[s], t_bounds[s + 1]
                mpsum = psS.tile([2 * D, SSL], f32, tag="scat")
                first = True
                for pi, (pos_sb, vmap) in enumerate(passes):
                    for t in range(t_lo, t_hi):
                        # tensor_scalar keeps every non-scalar operand packed
                        # fp16, enabling the DVE 2x/4x fast modes.
                        oh = ohpool.tile([128, SSL], f16, tag="oh")
                        nc.vector.tensor_scalar(
                            oh[:],
                            iota_f[:],
                            pos_sb[:, t : t + 1],
                            None,
                            op0=mybir.AluOpType.is_equal,
                        )
                        v, tl = vmap[t]
                        nc.tensor.matmul(
                            mpsum[:],
                            lhsT=v[:, tl * 8 : (tl + 1) * 8],
                            rhs=oh[:],
                            start=first,
                            stop=(pi == 1 and t == t_hi - 1),
                        )
                        first = False
                nc.vector.tensor_copy(
                    MT_sb[0 : 2 * D, s * SSL : (s + 1) * SSL], mpsum[:]
                )

            # ---- MLP: y = sigmoid(tanh(M @ W1 + b1) @ W2 + b2) ----
            for s in range(NMSL):
                hpsum = psH.tile([H, MSL], f32, tag="h_ps")
                nc.tensor.matmul(
                    hpsum[:],
                    lhsT=W1_sb[:],
                    rhs=MT_sb[:, s * MSL : (s + 1) * MSL],
                    start=True,
                    stop=True,
                )
                h_sb = mlp_pool.tile([H, MSL], f16, tag="h_sb")
                nc.scalar.activation(
                    h_sb[:], hpsum[:], mybir.ActivationFunctionType.Tanh,
                    bias=b1_sb[:],
                )
                ypsum = psY.tile([1, MSL], f32, tag="y_ps")
                nc.tensor.matmul(
                    ypsum[:], lhsT=W2_sb[:], rhs=h_sb[:], start=True, stop=True
                )
                y_sb = mlp_pool.tile([1, MSL], f32, tag="y_sb")
                nc.scalar.activation(
                    y_sb[:], ypsum[:], mybir.ActivationFunctionType.Sigmoid,
                    bias=b2_sb[:],
                )
                nc.sync.dma_start(
                    out=y[:, s * MSL : (s + 1) * MSL], in_=y_sb[:]
                )

    nc.compile()
    return nc


def _get_sparse_nc(ts: tuple):
    if ts not in _cached_sparse:
        _cached_sparse[ts] = _build_sparse(ts)
    return _cached_sparse[ts]


def _extract_onehot(R: np.ndarray):
    """Return idx[E] with R[idx[k], k] == 1 if R is exactly one-hot per
    column (and zero elsewhere); None otherwise."""
    rows, cols = np.nonzero(R)
    if cols.size != E:
        return None
    cnt = np.bincount(cols, minlength=E)
    if cnt.max() != 1 or cnt.min() != 1:
        return None
    if not np.all(R[rows, cols] == 1.0):
        return None
    idx = np.empty(E, np.int64)
    idx[cols] = rows
    return idx


def _pass_counts(tgt: np.ndarray):
    order = np.argsort(tgt, kind="stable")
    bounds = np.searchsorted(tgt[order], np.arange(0, N + 1, SSL))
    cnts = np.diff(bounds).reshape(CORES, NSSL)
    return order, bounds, cnts


def _pack_pass(tgt, src, e, order, bounds, ts):
    """Sort edges by target node, shard by target-node range across cores,
    pad each SSL-node slab's edge list to ts[s] 128-slot tiles.

    Returns per_core list of (idx16, pos_f32, ew_f32)."""
    tgt_s = tgt[order]
    src_s = src[order]
    e_s = e[order]

    per_core = []
    for c in range(CORES):
        src_cols, pos_cols, ew_cols = [], [], []
        for s in range(NSSL):
            Ts = ts[s]
            g = c * NSSL + s
            lo, hi = bounds[g], bounds[g + 1]
            cnt = hi - lo
            pad = Ts * 128
            si = np.zeros(pad, np.int16)
            po = np.full(pad, -1.0, np.float32)
            ew = np.zeros(pad, np.float32)
            si[:cnt] = src_s[lo:hi]
            po[:cnt] = (tgt_s[lo:hi] - (c * NPC + s * SSL)).astype(np.float32)
            ew[:cnt] = e_s[lo:hi]
            # slot j = t*128 + p  ->  column t, partition p
            src_cols.append(si)
            pos_cols.append(po.reshape(Ts, 128).T)
            ew_cols.append(ew.reshape(Ts, 128).T)
        # dma_gather wrapped index layout: idx j at partition j % 16,
        # column j // 16, replicated across the 8 gpsimd cores.
        flat = np.concatenate(src_cols)
        idx16 = np.tile(flat.reshape(-1, 16).T, (8, 1))
        per_core.append(
            (
                np.ascontiguousarray(idx16),
                np.ascontiguousarray(np.concatenate(pos_cols, axis=1)),
                np.ascontiguousarray(np.concatenate(ew_cols, axis=1)),
            )
        )
    return per_core


def _prepare_sparse(X, e, idx_i, idx_o, W1, b1, W2, b2):
    X = np.ascontiguousarray(np.asarray(X, dtype=np.float32))
    e_flat = np.asarray(e, dtype=np.float32).reshape(-1)
    XT = np.ascontiguousarray(X.T)
    Xpad = np.zeros((N, 64), np.float32)
    Xpad[:, :D] = X

    # pass A: mi[n] = sum_{k: idx_i[k]=n} e[k] * X[idx_o[k]]
    ordA, bndA, cntA = _pass_counts(idx_i)
    # pass B: mo[n] = sum_{k: idx_o[k]=n} e[k] * X[idx_i[k]]
    ordB, bndB, cntB = _pass_counts(idx_o)
    ts = tuple(
        max(1, int(np.ceil(max(cntA[:, s].max(), cntB[:, s].max()) / 128)))
        for s in range(NSSL)
    )
    packA = _pack_pass(idx_i, idx_o, e_flat, ordA, bndA, ts)
    packB = _pack_pass(idx_o, idx_i, e_flat, ordB, bndB, ts)

    W1c = np.ascontiguousarray(np.asarray(W1, dtype=np.float16))
    b1c = np.ascontiguousarray(np.asarray(b1, dtype=np.float32).reshape(H, 1))
    W2c = np.ascontiguousarray(np.asarray(W2, dtype=np.float16).reshape(H, 1))
    b2c = np.ascontiguousarray(np.asarray(b2, dtype=np.float32).reshape(1, 1))

    in_maps = []
    for c in range(CORES):
        ia, pa, ea = packA[c]
        ib, pb, eb = packB[c]
        in_maps.append(
            {
                "Xpad": Xpad,
                "XTl": np.ascontiguousarray(
                    XT[:, c * NPC : (c + 1) * NPC].astype(np.float16)
                ),
                "idxA": ia,
                "metaA": np.ascontiguousarray(np.concatenate([pa, ea], axis=1)),
                "idxB": ib,
                "metaB": np.ascontiguousarray(np.concatenate([pb, eb], axis=1)),
                "W1": W1c,
                "b1": b1c,
                "W2": W2c,
                "b2": b2c,
            }
        )
    return ts, in_maps


def plan(inputs: dict):
    """Resolve the execution plan for these inputs.

    Returns (nc, in_maps, assemble) where assemble(results) -> full [N, 1]
    output. Falls back to the dense kernel when Ri/Ro are not one-hot."""
    Ri = np.asarray(inputs["Ri"], dtype=np.float32)
    Ro = np.asarray(inputs["Ro"], dtype=np.float32)
    idx_i = _extract_onehot(Ri)
    idx_o = _extract_onehot(Ro)
    if idx_i is not None and idx_o is not None:
        ts, in_maps = _prepare_sparse(
            inputs["X"], inputs["e"], idx_i, idx_o,
            inputs["W1"], inputs["b1"], inputs["W2"], inputs["b2"],
        )
        nc = _get_sparse_nc(ts)

        def assemble(results):
            return np.concatenate(
                [
                    np.asarray(results[c]["y"], dtype=np.float32).reshape(NPC)
                    for c in range(CORES)
                ]
            ).reshape(N, 1)

        return nc, in_maps, assemble

    # dense fallback
    in_maps = _prepare_in_maps(
        inputs["X"], inputs["e"], Ri, Ro,
        inputs["W1"], inputs["b1"], inputs["W2"], inputs["b2"],
    )
    nc = _get_nc()

    def assemble(results):
        return np.asarray(results[0]["y"], dtype=np.float32).reshape(N, 1)

    return nc, in_maps, assemble


def kernel(**inputs) -> np.ndarray:
    global _last_exec_time_ns
    from concourse import bass_utils

    nc, in_maps, assemble = plan(inputs)
    trace = os.environ.get("KERNEL_TRACE", "") == "1"
    res = bass_utils.run_bass_kernel_spmd(
        nc, in_maps, core_ids=list(range(CORES)), trace=trace
    )
    _last_exec_time_ns = res.exec_time_ns
    return assemble(res.results)


# --------------------------------------------------------------------------
# dense fallback path (original kernel, used only if Ri/Ro aren't one-hot)
# --------------------------------------------------------------------------

def _build(collective: bool = True, phases: int = 4, r_dtype: str = "float16"):
    # phases: 1=gather only, 2=+scatter, 3=+allreduce, 4=+mlp (full)
    import concourse.bass as bass
    import concourse.bacc as bacc
    import concourse.mybir as mybir
    import concourse.tile as tile

    f32 = mybir.dt.float32
    f16 = mybir.dt.float16
    fR = getattr(mybir.dt, r_dtype)

    nc = bacc.Bacc(
        "TRN2",
        target_bir_lowering=False,
        debug=False,
        num_devices=CORES if collective else 1,
    )

    Ri_nat = nc.dram_tensor("Ri_nat", [N, ESH], fR, kind="ExternalInput").ap()
    Ro_nat = nc.dram_tensor("Ro_nat", [N, ESH], fR, kind="ExternalInput").ap()
    RiT = nc.dram_tensor("RiT", [ESH, N], fR, kind="ExternalInput").ap()
    RoT = nc.dram_tensor("RoT", [ESH, N], fR, kind="ExternalInput").ap()
    Xg = nc.dram_tensor("Xg", [128, NCH * D], f16, kind="ExternalInput").ap()
    XT = nc.dram_tensor("XT", [D, N], f32, kind="ExternalInput").ap()
    esh = nc.dram_tensor("esh", [128, ECH], f32, kind="ExternalInput").ap()
    W1 = nc.dram_tensor("W1", [3 * D, H], f16, kind="ExternalInput").ap()
    b1 = nc.dram_tensor("b1", [H, 1], f32, kind="ExternalInput").ap()
    W2 = nc.dram_tensor("W2", [H, 1], f32, kind="ExternalInput").ap()
    b2 = nc.dram_tensor("b2", [1, 1], f32, kind="ExternalInput").ap()
    y = nc.dram_tensor("y", [1, N], f32, kind="ExternalOutput").ap()

    with tile.TileContext(nc) as tc:
        with (
            tc.tile_pool(name="const", bufs=1) as const,
            tc.tile_pool(name="gslab", bufs=3) as gslab_pool,
            tc.tile_pool(name="sslab", bufs=2) as sslab_pool,
            tc.tile_pool(name="small", bufs=1) as small,
            tc.tile_pool(name="mlp", bufs=2) as mlp_pool,
            tc.tile_pool(name="psA", bufs=2, space="PSUM") as psA,
            tc.tile_pool(name="psB", bufs=2, space="PSUM") as psB,
            tc.tile_pool(name="dram", bufs=1, space="DRAM") as dram,
        ):
            Xg_sb = const.tile([128, NCH * D], f16)
            nc.sync.dma_start(out=Xg_sb[:], in_=Xg[:])
            e_sb = const.tile([128, ECH], f32)
            nc.sync.dma_start(out=e_sb[:], in_=esh[:])
            W1_sb = const.tile([3 * D, H], f32)
            nc.sync.dma_start(out=W1_sb[:], in_=W1[:])
            b1_sb = const.tile([H, 1], f32)
            nc.sync.dma_start(out=b1_sb[:], in_=b1[:])
            W2_sb = const.tile([H, 1], f16)
            nc.sync.dma_start(out=W2_sb[:], in_=W2[:])
            b2_sb = const.tile([1, 1], f32)
            nc.sync.dma_start(out=b2_sb[:], in_=b2[:])

            MT_sb = small.tile([3 * D, N], f32)
            nc.sync.dma_start(out=MT_sb[2 * D : 3 * D, :], in_=XT[:])

            bvi = small.tile([128, ECH * 12], f16)
            bvo = small.tile([128, ECH * 12], f16)
            nc.vector.memset(bvi[:], 0.0)
            nc.vector.memset(bvo[:], 0.0)

            for Rnat, dst, col0, acc_tag in (
                (Ri_nat, bvo, 4, "bacc_i"),
                (Ro_nat, bvi, 0, "bacc_o"),
            ):
                bacc_t = small.tile([128, ECH * D], f32, tag=acc_tag)
                for nch in range(NCH):
                    slab = gslab_pool.tile([128, ESH], fR, tag="gs")
                    nc.sync.dma_start(
                        out=slab[:], in_=Rnat[nch * 128 : (nch + 1) * 128, :]
                    )
                    bpsum = psA.tile([128, ECH * D], f32, tag="gather_ps")
                    for ech in range(ECH):
                        nc.tensor.matmul(
                            bpsum[:, ech * D : (ech + 1) * D],
                            lhsT=slab[:, ech * 128 : (ech + 1) * 128],
                            rhs=Xg_sb[:, nch * D : (nch + 1) * D],
                            start=True,
                            stop=True,
                        )
                    if nch == 0:
                        nc.vector.tensor_copy(bacc_t[:], bpsum[:])
                    else:
                        nc.vector.tensor_add(bacc_t[:], bacc_t[:], bpsum[:])
                for ech in range(ECH):
                    nc.vector.tensor_scalar_mul(
                        dst[:, ech * 12 + col0 : ech * 12 + col0 + D],
                        bacc_t[:, ech * D : (ech + 1) * D],
                        e_sb[:, ech : ech + 1],
                    )

            RiT3 = RiT.rearrange("(ec p) n -> p ec n", p=128)
            RoT3 = RoT.rearrange("(ec p) n -> p ec n", p=128)
            for ns in range(NS if phases >= 2 else 0):
                mpsum = psB.tile([3 * D, NSLAB], f32, tag="scat_ps")
                first = True
                for RT3, bv, stag in ((RiT3, bvi, "ssi"), (RoT3, bvo, "sso")):
                    tslab = sslab_pool.tile([128, ECH, NSLAB], fR, tag=stag)
                    nc.sync.dma_start(
                        out=tslab[:],
                        in_=RT3[:, :, ns * NSLAB : (ns + 1) * NSLAB],
                    )
                    for ech in range(ECH):
                        nc.tensor.matmul(
                            mpsum[:],
                            lhsT=bv[:, ech * 12 : (ech + 1) * 12],
                            rhs=tslab[:, ech, :],
                            start=first,
                            stop=(bv is bvo and ech == ECH - 1),
                        )
                        first = False
                nc.vector.tensor_copy(
                    MT_sb[0 : 2 * D, ns * NSLAB : (ns + 1) * NSLAB],
                    mpsum[0 : 2 * D, :],
                )

            if collective and phases >= 3:
                ar_in = dram.tile([2 * D, N], f32)
                ar_out = dram.tile([2 * D, N], f32, addr_space="Shared")
                nc.gpsimd.dma_start(out=ar_in[:], in_=MT_sb[0 : 2 * D, :])
                nc.gpsimd.collective_compute(
                    "AllReduce",
                    mybir.AluOpType.add,
                    replica_groups=[list(range(CORES))],
                    ins=[ar_in.opt()],
                    outs=[ar_out.opt()],
                )
                nc.gpsimd.dma_start(out=MT_sb[0 : 2 * D, :], in_=ar_out[:])

            for ns in range(NS if phases >= 4 else 0):
                hpsum = psB.tile([H, NSLAB], f32, tag="h_ps")
                nc.tensor.matmul(
                    hpsum[:],
                    lhsT=W1_sb[:],
                    rhs=MT_sb[:, ns * NSLAB : (ns + 1) * NSLAB],
                    start=True,
                    stop=True,
                )
                h_sb = mlp_pool.tile([H, NSLAB], f32, tag="h_sb")
                nc.scalar.activation(
                    h_sb[:], hpsum[:], mybir.ActivationFunctionType.Tanh,
                    bias=b1_sb[:],
                )
                ypsum = psB.tile([1, NSLAB], f32, tag="y_ps")
                nc.tensor.matmul(
                    ypsum[:], lhsT=W2_sb[:], rhs=h_sb[:], start=True, stop=True
                )
                y_sb = mlp_pool.tile([1, NSLAB], f32, tag="y_sb")
                nc.scalar.activation(
                    y_sb[:], ypsum[:], mybir.ActivationFunctionType.Sigmoid,
                    bias=b2_sb[:],
                )
                nc.sync.dma_start(
                    out=y[:, ns * NSLAB : (ns + 1) * NSLAB], in_=y_sb[:]
                )

    nc.compile()
    return nc


def _get_nc(r_dtype: str = "float16"):
    if r_dtype not in _cached:
        _cached[r_dtype] = _build(r_dtype=r_dtype)
    return _cached[r_dtype]


def _prepare_in_maps(X, e, Ri, Ro, W1, b1, W2, b2, r_dtype: str = "float16"):
    X = np.asarray(X, dtype=np.float32)
    e = np.asarray(e, dtype=np.float32)
    W1 = np.asarray(W1, dtype=np.float32)
    b1 = np.asarray(b1, dtype=np.float32)
    W2 = np.asarray(W2, dtype=np.float32)
    b2 = np.asarray(b2, dtype=np.float32)

    rdt = np.float16
    Ri16 = np.asarray(Ri, dtype=np.float32).astype(rdt)
    Ro16 = np.asarray(Ro, dtype=np.float32).astype(rdt)
    RiT16 = np.ascontiguousarray(Ri16.T)
    RoT16 = np.ascontiguousarray(Ro16.T)

    X16 = X.astype(np.float16)
    Xg = np.ascontiguousarray(
        X16.reshape(NCH, 128, D).transpose(1, 0, 2).reshape(128, NCH * D)
    )
    XT = np.ascontiguousarray(X.T)

    b1c = np.ascontiguousarray(b1.reshape(H, 1))
    b2c = np.ascontiguousarray(b2.reshape(1, 1))
    W1c = np.ascontiguousarray(W1)
    W2c = np.ascontiguousarray(W2.reshape(H, 1))

    in_maps = []
    for c in range(CORES):
        sh = slice(c * ESH, (c + 1) * ESH)
        e_c = np.ascontiguousarray(
            e.reshape(-1)[sh].reshape(ECH, 128).T
        ).astype(np.float32)
        in_maps.append(
            {
                "Ri_nat": np.ascontiguousarray(Ri16[:, sh]),
                "Ro_nat": np.ascontiguousarray(Ro16[:, sh]),
                "RiT": RiT16[sh],
                "RoT": RoT16[sh],
                "Xg": Xg,
                "XT": XT,
                "esh": e_c,
                "W1": W1c,
                "b1": b1c,
                "W2": W2c,
                "b2": b2c,
            }
        )
    return in_maps
